# revision 4
# baseline (speedup 1.0000x reference)
"""Multi-head attention (b=2, n=2048, d_model=1024, H=16, d_k=d_v=64) on 8
Trainium2 NeuronCores — v5.

v4 + deeper HAM-aware scheduling.  The core-wide activity throttle halves
the clock whenever the PE shows idle gaps, and the ACT engine (exp ~1.1us
per k-tile) otherwise outpaces the PE (~1.05us), so v5:

  * shrinks phase A to the minimum head 0 needs (kproj chunk 0 + the j=0
    q-halves); kproj chunks 1-3 and the j=1 q-halves become head-0 fillers
  * hoists all filler DMAs ahead of their compute (late x chunks ride the
    scalar queue so they don't delay the V DMAs on the sync queue)
  * pads heads with little filler by splitting AV matmuls into N=256 pairs
    (real work, ~+0.3us/k-tile) so the PE stays the pacing engine
  * tail out-projection alternates PSUM rings (fill/avs) for a 4-deep pipe

PSUM: st [128,1024] x2 (4 banks) + avs [128,512] x2 + fill [128,512] x2.
Exp pipeline depth 2.
"""

import numpy as np
import ml_dtypes
from contextlib import ExitStack

import concourse.bass as bass
import concourse.mybir as mybir
import concourse.tile as tile
from concourse import bacc
from concourse.bass_utils import run_bass_kernel_spmd

F32 = mybir.dt.float32
BF16 = mybir.dt.bfloat16
EXP = mybir.ActivationFunctionType.Exp
IDENT = mybir.ActivationFunctionType.Identity
ADD = mybir.AluOpType.add
MULT = mybir.AluOpType.mult

D_MODEL = 1024
H = 16
DK = 64
B = 2
N = 2048           # nq = nk
G = 4              # head groups (cores per batch)
HG = H // G        # heads per group = 4
DG = HG * DK       # 256 group dims
KT = 8             # D_MODEL / 128 contraction tiles
NKT = N // 128     # 16 k-tiles in attention
QC = 1024          # attention q-chunk
NCH = N // QC      # 2 chunks
P = 128
DEPTH = 3          # exp software-pipeline depth

_PROGRAM = None


def _build_program():
    nc = bacc.Bacc("TRN2", target_bir_lowering=False, debug=False, num_devices=8)

    xqT = nc.dram_tensor("xqT", [4, P, KT, 512], BF16, kind="ExternalInput").ap()
    xkT = nc.dram_tensor("xkT", [4, P, KT, 512], BF16, kind="ExternalInput").ap()
    xvT = nc.dram_tensor("xvT", [NKT, P, KT, P], BF16, kind="ExternalInput").ap()
    wqT = nc.dram_tensor("wqT", [P, KT, DG], BF16, kind="ExternalInput").ap()
    wkT = nc.dram_tensor("wkT", [P, KT, DG], BF16, kind="ExternalInput").ap()
    wvT = nc.dram_tensor("wvT", [P, KT, DG], BF16, kind="ExternalInput").ap()
    woT = nc.dram_tensor("woT", [P, 2, D_MODEL], BF16, kind="ExternalInput").ap()
    bq_d = nc.dram_tensor("bq_s", [DG], F32, kind="ExternalInput").ap()
    bk_d = nc.dram_tensor("bk_s", [DG], F32, kind="ExternalInput").ap()
    bv_d = nc.dram_tensor("bv_s", [512], BF16, kind="ExternalInput").ap()
    yT_d = nc.dram_tensor("yT", [D_MODEL, N], F32, kind="ExternalOutput").ap()

    bq_v = bq_d.rearrange("(j p) -> p j", p=P)        # [128, 2]
    bk_v = bk_d.rearrange("(j p) -> p j", p=P)

    with tile.TileContext(nc) as tc:
        with ExitStack() as ctx:
            const = ctx.enter_context(tc.tile_pool(name="const", bufs=1))
            xin = ctx.enter_context(tc.tile_pool(name="xin", bufs=6))
            xvp = ctx.enter_context(tc.tile_pool(name="xvp", bufs=3))
            work = ctx.enter_context(tc.tile_pool(name="work", bufs=2))
            atp = ctx.enter_context(tc.tile_pool(name="atp", bufs=5))
            smal = ctx.enter_context(tc.tile_pool(name="smal", bufs=3))
            psum = ctx.enter_context(tc.tile_pool(name="psum", bufs=1, space="PSUM"))

            wq_sb = const.tile([P, KT, DG], BF16, tag="wq")
            wk_sb = const.tile([P, KT, DG], BF16, tag="wk")
            wv_sb = const.tile([P, KT, DG], BF16, tag="wv")
            wo_sb = const.tile([P, 2, D_MODEL], BF16, tag="wo")
            nc.scalar.dma_start(wk_sb[:], wkT)
            nc.scalar.dma_start(wq_sb[:], wqT)
            nc.scalar.dma_start(wv_sb[:], wvT)
            nc.scalar.dma_start(wo_sb[:], woT)
            bq_sb = const.tile([P, 2], F32, tag="bq")
            bk_sb = const.tile([P, 2], F32, tag="bk")
            nc.scalar.dma_start(bq_sb[:], bq_v)
            nc.scalar.dma_start(bk_sb[:], bk_v)
            bv_sb = const.tile([1, 512], BF16, tag="bv")   # [bv|bv] paired
            nc.scalar.dma_start(bv_sb[:], bv_d[None, :])
            ones_sb = const.tile([1, P], BF16, tag="ones")
            nc.vector.memset(ones_sb[:], 1.0)

            kt_sb = const.tile([P, 2, N], BF16, tag="kt")           # K^T
            v_sb = const.tile([P, NKT, HG, DK + 1], BF16, tag="v")  # [V_h | 1]
            nc.vector.memset(v_sb[:, :, :, DK], 1.0)

            def xk_dma(c4, eng, split=1):
                xk = xin.tile([P, KT, 512], BF16, tag="xchunk", name=f"xk_{c4}")
                w = KT // split
                for piece in range(split):
                    eng.dma_start(xk[:, w * piece:w * piece + w, :],
                                  xkT[c4, :, w * piece:w * piece + w, :])
                return xk

            def xq_dma(c, qh, eng):
                xq = xin.tile([P, KT, 512], BF16, tag="xchunk",
                              name=f"xq_{c}_{qh}")
                eng.dma_start(xq[:], xqT[c * 2 + qh])
                return xq

            def kproj_mms(c4, xk, on_act):
                for j in range(2):
                    ps = psum.tile([P, 512], F32, tag="fill", bufs=2,
                                   name=f"kps_{c4}_{j}")
                    for k in range(KT):
                        nc.tensor.matmul(
                            ps[:], wk_sb[:, k, j * P:(j + 1) * P], xk[:, k, :],
                            start=(k == 0), stop=(k == KT - 1))
                    out = kt_sb[:, j, c4 * 512:(c4 + 1) * 512]
                    if on_act:
                        nc.scalar.activation(out, ps[:], IDENT,
                                             bias=bk_sb[:, j:j + 1])
                    else:
                        nc.vector.tensor_tensor(
                            out, ps[:],
                            bk_sb[:, j, None].to_broadcast((P, 512)), ADD)

            def qproj_mms(c, qt, qh, j, xq, on_act):
                ps = psum.tile([P, 512], F32, tag="fill", bufs=2,
                               name=f"qps_{c}_{qh}_{j}")
                for k in range(KT):
                    nc.tensor.matmul(
                        ps[:], wq_sb[:, k, j * P:(j + 1) * P], xq[:, k, :],
                        start=(k == 0), stop=(k == KT - 1))
                out = qt[:, j, qh * 512:(qh + 1) * 512]
                if on_act:
                    nc.scalar.activation(out, ps[:], IDENT,
                                         bias=bq_sb[:, j:j + 1])
                else:
                    nc.vector.tensor_tensor(
                        out, ps[:],
                        bq_sb[:, j, None].to_broadcast((P, 512)), ADD)

            def xv_dma(np2):
                xv = xvp.tile([P, 2, KT, P], BF16, tag="xv", name=f"xv_{np2}")
                nc.sync.dma_start(xv[:, 0], xvT[np2 * 2])
                nc.sync.dma_start(xv[:, 1], xvT[np2 * 2 + 1])
                return xv

            def v_proj(np2, xv=None):
                # two n-tiles (np2*2, np2*2+1) share one [128,512] psum
                if xv is None:
                    xv = xv_dma(np2)
                ps = psum.tile([P, 512], F32, tag="fill", bufs=2,
                               name=f"vps_{np2}")
                for half in range(2):
                    for k in range(KT):
                        nc.tensor.matmul(
                            ps[:, half * DG:(half + 1) * DG],
                            xv[:, half, k, :], wv_sb[:, k, :],
                            start=(k == 0), stop=False)
                    nc.tensor.matmul(
                        ps[:, half * DG:(half + 1) * DG], ones_sb[:],
                        bv_sb[:, half * DG:(half + 1) * DG],
                        start=False, stop=True)
                nc.vector.tensor_copy(
                    v_sb[:, np2 * 2:np2 * 2 + 2, :, 0:DK],
                    ps[:].rearrange("p (n h d) -> p n h d", n=2, h=HG))

            def oproj_unit(c, o_sb, m, qh, act_copy, ptag="fill",
                           dma_eng=None):
                yps = psum.tile([P, 512], F32, tag=ptag, bufs=2,
                                name=f"yps_{c}_{m}_{qh}")
                for j in range(2):
                    nc.tensor.matmul(
                        yps[:], wo_sb[:, j, m * P:(m + 1) * P],
                        o_sb[:, j, qh * 512:(qh + 1) * 512],
                        start=(j == 0), stop=(j == 1))
                y_sb = smal.tile([P, 512], F32, tag="y", bufs=6,
                                 name=f"y_{c}_{m}_{qh}")
                if act_copy:
                    nc.scalar.copy(y_sb[:], yps[:])
                else:
                    nc.vector.tensor_copy(y_sb[:], yps[:])
                (dma_eng or nc.gpsimd).dma_start(
                    yT_d[m * P:(m + 1) * P,
                         c * QC + qh * 512:c * QC + (qh + 1) * 512],
                    y_sb[:])

            def denom_qh(avs, o_sb, h, qh, tag):
                """1/Z broadcast + normalize for (head h, q-half qh)."""
                p0 = (h % 2) * 64
                j = h // 2
                zr = smal.tile([1, 512], F32, tag="zr", name=f"zr_{tag}_{qh}")
                nc.vector.tensor_copy(zr[:], avs[qh][DK:DK + 1, :])
                rz = smal.tile([1, 512], F32, tag="rz", name=f"rz_{tag}_{qh}")
                nc.vector.reciprocal_approx_fast(rz[:], zr[:])
                rz16 = smal.tile([1, 512], BF16, tag="rz16",
                                 name=f"rz16_{tag}_{qh}")
                nc.vector.tensor_copy(rz16[:], rz[:])
                rzb = psum.tile([64, 512], F32, tag="fill", bufs=2,
                                name=f"rzb_{tag}_{qh}")
                nc.tensor.matmul(rzb[:], ones_sb[:, 0:64], rz16[:],
                                 start=True, stop=True)
                rzs = smal.tile([64, 512], BF16, tag="rzs",
                                name=f"rzs_{tag}_{qh}")
                nc.vector.tensor_copy(rzs[:], rzb[:])
                nc.vector.tensor_tensor(
                    o_sb[p0:p0 + 64, j, qh * 512:(qh + 1) * 512],
                    avs[qh][0:DK, :], rzs[:], MULT)

            # ---- phase A: only what head 0 of chunk 0 needs up front ----
            qts = {c: work.tile([P, 2, QC], BF16, tag="qt", name=f"qt_{c}")
                   for c in range(NCH)}
            xk0 = xk_dma(0, nc.sync, split=4)
            xq0 = {0: xq_dma(0, 0, nc.sync)}
            kproj_mms(0, xk0, on_act=True)
            xv0 = xv_dma(0)
            xq0[1] = xq_dma(0, 1, nc.sync)
            for qh in range(2):
                qproj_mms(0, qts[0], qh, 0, xq0[qh], on_act=True)
            # late x chunks ride the scalar queue (weights are done by now)
            # so they don't delay the V DMAs on the sync queue
            xks = {c4: xk_dma(c4, nc.scalar) for c4 in range(1, 4)}

            o_sbs = {c: work.tile([P, 2, QC], BF16, tag="o", name=f"o_{c}")
                     for c in range(NCH)}

            for c in range(NCH):
                qt = qts[c]
                o_sb = o_sbs[c]

                # filler schedule: fillers[(h, kt)] -> list of thunks
                fillers = {}
                if c == 0:
                    # vproj pair 0 pre-loop; pair np2 at (h0, 2*np2-2)
                    # stays 4+ n-tiles ahead of AV(kt-DEPTH)
                    v_proj(0, xv=xv0)
                    for np2 in range(1, 8):
                        fillers.setdefault((0, np2 * 2 - 2), []).append(
                            lambda n=np2: v_proj(n))
                    # kproj chunks 1-3 in head 0, ahead of scores kt>=4c4
                    for c4 in range(1, 4):
                        fillers.setdefault((0, 2 * c4 - 1), []).append(
                            lambda cc=c4: kproj_mms(cc, xks[cc], False))
                    # q chunk 0 j=1 halves (first needed by head 2)
                    for qh in range(2):
                        fillers.setdefault((0, 7 + 2 * qh), []).append(
                            lambda q=qh: qproj_mms(0, qts[0], q, 1, xq0[q],
                                                   False))
                    # next chunk's q projection spread over heads 1-2
                    xq1 = {qh: xq_dma(1, qh, nc.gpsimd) for qh in range(2)}
                    for i in range(4):
                        qh, j = i % 2, i // 2
                        fillers.setdefault((1 + j, qh * 8 + 3), []).append(
                            lambda q=qh, jj=j: qproj_mms(1, qts[1], q, jj,
                                                         xq1[q], False))
                else:
                    # chunk 0's out-projection spread over all heads
                    for i in range(16):
                        m, qh = i % 8, i // 8
                        fillers.setdefault((i // 4, (i % 4) * 4 + 2),
                                           []).append(
                            lambda mm=m, q=qh: oproj_unit(0, o_sbs[0], mm, q,
                                                          act_copy=False))

                pending = []  # denom thunks still to issue
                for h in range(HG):
                    # heads with little filler pad the PE with split (N=256)
                    # AV matmuls so the PE stays ahead of the ACT engine
                    pad = not (c == 0 and h == 0)
                    p0 = (h % 2) * 64
                    j = h // 2
                    avs = [psum.tile([DK + 1, 512], F32, tag="avs", bufs=2,
                                     name=f"av_{c}_{h}_{qh}")
                           for qh in range(2)]
                    ats = {}
                    halves = 2 if pad else 1
                    w = 512 // halves
                    for kt in range(NKT + DEPTH):
                        if kt < NKT:
                            st = psum.tile([P, QC], F32, tag="st", bufs=2,
                                           name=f"st_{c}_{h}_{kt}")
                            # padded heads split scores into N=256 singles:
                            # extra per-matmul overhead is deliberate PE
                            # filler that keeps the PE ahead of ACT
                            for qh in range(2):
                                for hf in range(halves):
                                    nc.tensor.matmul(
                                        st[:, qh * 512 + hf * w:
                                           qh * 512 + (hf + 1) * w],
                                        kt_sb[p0:p0 + 64, j,
                                              kt * P:(kt + 1) * P],
                                        qt[p0:p0 + 64, j,
                                           qh * 512 + hf * w:
                                           qh * 512 + (hf + 1) * w],
                                        start=True, stop=True)
                            at = atp.tile([P, QC], BF16, tag="at",
                                          name=f"at_{c}_{h}_{kt}")
                            nc.scalar.activation(at[:], st[:], EXP, scale=0.125)
                            ats[kt] = at
                        if kt in (0, 1) and pending:
                            pending.pop(0)()
                        for u in fillers.pop((h, kt), []):
                            u()
                        if kt >= DEPTH:
                            at = ats.pop(kt - DEPTH)
                            for qh in range(2):
                                nc.tensor.matmul(
                                    avs[qh], v_sb[:, kt - DEPTH, h, :],
                                    at[:, qh * 512:(qh + 1) * 512],
                                    start=(kt - DEPTH == 0),
                                    stop=(kt - DEPTH == NKT - 1))
                    for qh in range(2):
                        pending.append(
                            (lambda a=avs, hh=h, q=qh, t=f"{c}_{h}":
                             denom_qh(a, o_sb, hh, q, t)))

                while pending:
                    pending.pop(0)()

            # tail: chunk 1's out-projection (ACT is free now; alternate
            # copies between DVE/ACT and psums between fill/avs rings)
            tail_engs = [nc.gpsimd, nc.sync, nc.scalar]
            for qh in range(2):
                for m in range(8):
                    oproj_unit(1, o_sbs[1], m, qh, act_copy=(m % 2 == 1),
                               ptag="fill" if m % 2 == 0 else "avs",
                               dma_eng=tail_engs[(qh * 8 + m) % 3])

    nc.compile()
    return nc


def get_program():
    global _PROGRAM
    if _PROGRAM is None:
        _PROGRAM = _build_program()
    return _PROGRAM


def _tile_xT(x, nchunk, width):
    xt = np.ascontiguousarray(x.T)                      # [1024, n]
    return np.ascontiguousarray(
        xt.reshape(KT, P, nchunk, width).transpose(2, 1, 0, 3))


def _tile_w(w_rows):
    return np.ascontiguousarray(
        w_rows.T.reshape(KT, P, DG).transpose(1, 0, 2)).astype(ml_dtypes.bfloat16)


def make_in_maps(queries, keys, values, Wq, bq, Wk, bk, Wv, bv, Wo, bo):
    """Build per-core input dicts. Core c handles batch c//4, head group c%4."""
    bf16 = ml_dtypes.bfloat16
    xT = {}
    for ib in range(B):
        xT[ib] = (
            _tile_xT(np.asarray(queries[ib]).astype(bf16), 4, 512),
            _tile_xT(np.asarray(keys[ib]).astype(bf16), 4, 512),
            _tile_xT(np.asarray(values[ib]).astype(bf16), NKT, P),
        )
    in_maps = []
    for core in range(8):
        ib, g = core // G, core % G
        sl = slice(g * DG, (g + 1) * DG)
        in_maps.append({
            "xqT": xT[ib][0], "xkT": xT[ib][1], "xvT": xT[ib][2],
            "wqT": _tile_w(Wq[sl, :]),
            "wkT": _tile_w(Wk[sl, :]),
            "wvT": _tile_w(Wv[sl, :]),
            "woT": np.ascontiguousarray(
                Wo[:, sl].T.reshape(2, P, D_MODEL).transpose(1, 0, 2)
            ).astype(bf16),
            "bq_s": np.ascontiguousarray(bq[sl]).astype(np.float32),
            "bk_s": np.ascontiguousarray(bk[sl]).astype(np.float32),
            "bv_s": np.tile(np.asarray(bv[sl]), 2).astype(bf16),
        })
    return in_maps


def gather_output(results, bo):
    out = np.zeros((B, N, D_MODEL), np.float32)
    for core in range(8):
        out[core // G] += results[core]["yT"].T
    out += bo[None, None, :].astype(np.float32)
    return out


def _run(inputs, trace=False, **spmd_kwargs):
    nc = get_program()
    in_maps = make_in_maps(**inputs)
    res = run_bass_kernel_spmd(nc, in_maps, core_ids=list(range(8)),
                               trace=trace, **spmd_kwargs)
    return gather_output(res.results, inputs["bo"]), res


def kernel(**inputs) -> np.ndarray:
    out, _ = _run(inputs, trace=False)
    return out
